# revision 1
# baseline (speedup 1.0000x reference)
"""Int8 Llama MLP (W8A8) on 8 Trainium2 NeuronCores.

Strategy: tensor-parallel over the intermediate dim I (11008 -> 1376/core).
Each core computes, for all 8192 tokens:
    gT  = gate_wT_shard.T-contraction vs quantized-x  (PSUM f32, exact int math in bf16)
    uT  = same for up
    hT  = quantize(silu_f16(gT*gs) * (uT*us) / ds_in)  as bf16 integers
    outT_partial[H, T] f32 = down_wT_shard-contraction vs hT, scaled
Host: quantize+transpose+tile inputs (exact int8 -> bf16), sum the 8 partial
outputs, transpose back to [T, H].

All matmuls run with int-valued bf16 operands: products are exact in the f32
PSUM accumulation; only the >2^24 running-sum rounding differs from the int32
reference (rel err ~1e-6).
"""

import numpy as np
import ml_dtypes

import concourse.bass as bass
import concourse.mybir as mybir
import concourse.tile as tile
from concourse import bacc
from concourse.bass_utils import run_bass_kernel_spmd

T, H, I = 8192, 4096, 11008
NCORES = 8
IP = 11264                 # I zero-padded to a multiple of 8*128
ISH = IP // NCORES         # 1408 intermediate rows per core
NI = ISH // 128            # 11 partition tiles of the I-shard
KO = H // 128              # 32 k-chunks for gate/up contraction
TB = 512                   # token block (matmul free dim)
NB = T // TB               # 16 token blocks
NJ = H // 128              # 32 output H-tiles for down

F32 = mybir.dt.float32
F16 = mybir.dt.float16
BF16 = mybir.dt.bfloat16
MAGIC = 12582912.0         # 1.5 * 2^23: float32 round-to-nearest-even trick

_prog_cache = {}


def _build_program(share_x: bool, gsc: float, usc_over_dis: float, dsc: float):
    key = (share_x, gsc, usc_over_dis, dsc)
    if key in _prog_cache:
        return _prog_cache[key]

    nc = bacc.Bacc(None)
    xq = nc.declare_dram_parameter("xq", [NB, 128, KO, TB], BF16, isOutput=False)
    if share_x:
        xq2 = xq
    else:
        xq2 = nc.declare_dram_parameter("xq2", [NB, 128, KO, TB], BF16, isOutput=False)
    wg = nc.declare_dram_parameter("wg", [NI, 128, KO, 128], BF16, isOutput=False)
    wu = nc.declare_dram_parameter("wu", [NI, 128, KO, 128], BF16, isOutput=False)
    wd = nc.declare_dram_parameter("wd", [128, NI, NJ, 128], BF16, isOutput=False)
    outT = nc.declare_dram_parameter("outT", [H, T], F32, isOutput=True)

    ACT = mybir.ActivationFunctionType
    ALU = mybir.AluOpType

    with tile.TileContext(nc) as tc:
        with (
            tc.tile_pool(name="pwd", bufs=1) as pwd,
            tc.tile_pool(name="px", bufs=2) as px,
            tc.tile_pool(name="pw", bufs=3) as pw,
            tc.tile_pool(name="pht", bufs=NI + 1) as pht,
            tc.tile_pool(name="ptmp", bufs=2) as ptmp,
            tc.tile_pool(name="pout", bufs=2) as pout,
            tc.tile_pool(name="psg", bufs=2, space="PSUM") as psg,
            tc.tile_pool(name="psu", bufs=2, space="PSUM") as psu,
            tc.tile_pool(name="psd", bufs=2, space="PSUM") as psd,
        ):
            wd_sb = pwd.tile([128, NI, NJ, 128], BF16)
            nc.gpsimd.dma_start(wd_sb[:], wd[:])

            for b in range(NB):
                x_sb = px.tile([128, KO, TB], BF16, tag="x")
                nc.sync.dma_start(x_sb[:], xq[b])
                if share_x:
                    x2_sb = x_sb
                else:
                    x2_sb = px.tile([128, KO, TB], BF16, tag="x2")
                    nc.sync.dma_start(x2_sb[:], xq2[b])

                ht_tiles = []
                for i in range(NI):
                    wg_sb = pw.tile([128, KO, 128], BF16, tag="w")
                    nc.sync.dma_start(wg_sb[:], wg[i])
                    wu_sb = pw.tile([128, KO, 128], BF16, tag="w")
                    nc.sync.dma_start(wu_sb[:], wu[i])

                    g_ps = psg.tile([128, TB], F32)
                    for ko in range(KO):
                        nc.tensor.matmul(g_ps[:], wg_sb[:, ko, :], x_sb[:, ko, :],
                                         start=(ko == 0), stop=(ko == KO - 1))
                    u_ps = psu.tile([128, TB], F32)
                    for ko in range(KO):
                        nc.tensor.matmul(u_ps[:], wu_sb[:, ko, :], x2_sb[:, ko, :],
                                         start=(ko == 0), stop=(ko == KO - 1))

                    # hidden = silu(f16(g*gsc)) * (u*usc/dis), then round+clip to int8
                    t16 = ptmp.tile([128, TB], F16, tag="t16")
                    nc.scalar.activation(t16[:], g_ps[:], ACT.Copy, scale=gsc)
                    s16 = ptmp.tile([128, TB], F16, tag="s16")
                    nc.scalar.activation(s16[:], t16[:], ACT.Sigmoid)
                    sl16 = ptmp.tile([128, TB], F16, tag="sl16")
                    nc.vector.tensor_tensor(sl16[:], t16[:], s16[:], ALU.mult)
                    h32 = ptmp.tile([128, TB], F32, tag="h32")
                    nc.vector.scalar_tensor_tensor(h32[:], u_ps[:], usc_over_dis,
                                                   sl16[:], ALU.mult, ALU.mult)
                    # clamp to (-128.49, 127.49) pre-round: keeps magic-add in
                    # exact range and matches round-then-clip on boundaries
                    c32 = ptmp.tile([128, TB], F32, tag="c32")
                    nc.vector.tensor_scalar(c32[:], h32[:], -128.49, 127.49,
                                            ALU.max, ALU.min)
                    ht_i = pht.tile([128, TB], BF16, tag="ht")
                    nc.vector.tensor_scalar(ht_i[:], c32[:], MAGIC, MAGIC,
                                            ALU.add, ALU.subtract)
                    ht_tiles.append(ht_i)

                for j in range(NJ):
                    d_ps = psd.tile([128, TB], F32)
                    for k in range(NI):
                        nc.tensor.matmul(d_ps[:], wd_sb[:, k, j, :], ht_tiles[k][:],
                                         start=(k == 0), stop=(k == NI - 1))
                    o_sb = pout.tile([128, TB], F32, tag="o")
                    nc.scalar.activation(o_sb[:], d_ps[:], ACT.Copy, scale=dsc)
                    nc.sync.dma_start(outT[j * 128:(j + 1) * 128, b * TB:(b + 1) * TB],
                                      o_sb[:])

    nc.finalize()
    _prog_cache[key] = nc
    return nc


def _quant_tile_x(x: np.ndarray, scale: float) -> np.ndarray:
    """clip(round(x/scale)) -> tiled [NB, 128, KO, TB] bf16 (exact ints)."""
    q = np.clip(np.round(x / np.float32(scale)), -128, 127).astype(np.float32)
    return np.ascontiguousarray(
        q.reshape(NB, TB, KO, 128).transpose(0, 3, 2, 1)
    ).astype(ml_dtypes.bfloat16)


def _prepare_in_maps(x, gate_w, up_w, down_w, gis, uis, share_x):
    xq = _quant_tile_x(np.asarray(x, np.float32), gis)
    xq2 = None if share_x else _quant_tile_x(np.asarray(x, np.float32), uis)

    # zero-pad I (11008 -> 11264): padded gate/up rows give hidden=0 and the
    # padded down columns are 0, so the result is unchanged
    gw = np.zeros((IP, H), np.int8); gw[:I] = np.asarray(gate_w)
    uw = np.zeros((IP, H), np.int8); uw[:I] = np.asarray(up_w)
    dw = np.zeros((H, IP), np.int8); dw[:, :I] = np.asarray(down_w)

    in_maps = []
    for c in range(NCORES):
        i0, i1 = c * ISH, (c + 1) * ISH
        wg_c = np.ascontiguousarray(
            gw[i0:i1].reshape(NI, 128, KO, 128).transpose(0, 3, 2, 1)
        ).astype(ml_dtypes.bfloat16)
        wu_c = np.ascontiguousarray(
            uw[i0:i1].reshape(NI, 128, KO, 128).transpose(0, 3, 2, 1)
        ).astype(ml_dtypes.bfloat16)
        wd_c = np.ascontiguousarray(
            dw[:, i0:i1].reshape(NJ, 128, NI, 128).transpose(3, 2, 0, 1)
        ).astype(ml_dtypes.bfloat16)
        m = {"xq": xq, "wg": wg_c, "wu": wu_c, "wd": wd_c}
        if not share_x:
            m["xq2"] = xq2
        in_maps.append(m)
    return in_maps


def kernel(x, gate_w, up_w, down_w,
           gate_in_scale, gate_w_scale,
           up_in_scale, up_w_scale,
           down_in_scale, down_w_scale):
    gis = float(gate_in_scale)
    uis = float(up_in_scale)
    dis = float(down_in_scale)
    gsc = float(np.float32(gis) * np.float32(gate_w_scale))
    usc = float(np.float32(uis) * np.float32(up_w_scale))
    dsc = float(np.float32(dis) * np.float32(down_w_scale))
    share_x = (np.float32(gis) == np.float32(uis))

    nc = _build_program(share_x, gsc, usc / dis, dsc)
    in_maps = _prepare_in_maps(x, gate_w, up_w, down_w, gis, uis, share_x)

    res = run_bass_kernel_spmd(nc, in_maps, list(range(NCORES)))

    acc = res.results[0]["outT"]
    for c in range(1, NCORES):
        acc = acc + res.results[c]["outT"]
    return np.ascontiguousarray(acc.T)

